# revision 1
# baseline (speedup 1.0000x reference)
"""MoE MLP (top-1 routing) on 8 TRN2 NeuronCores.

Strategy (expert-parallel, per the sharding hint): the host computes the
router argmax and dispatches each token to its expert's core. Core e holds
w_fc[e]/w_proj[e] and runs the dense expert MLP
    y = (0.5*(x_fc + relu(x_fc)))^2 @ w_proj[e].T,   x_fc = x @ w_fc[e].T
on its gathered tokens in a transposed (feature-major) layout so no on-device
transposes are needed. Matmuls run as float32r (full-rate fp32 mode on the PE).

Tokens are processed in near-equal blocks of <=512. The x and y DRAM layouts
are block-major so each block transfer is one fully-contiguous-per-partition
DMA. The emission order software-pipelines blocks on the PE: GEMM2 of block
b-1 is issued between GEMM1 of block b and its activation epilogue, so the PE
never waits on the ACT/DVE chain.
"""

import contextlib
import numpy as np

import concourse.mybir as mybir
import concourse.tile as tile
from concourse import bacc
from concourse.bass_utils import run_bass_kernel_spmd

P = 128          # SBUF partitions / PE array dim
D = 1024         # model dim
E = 8            # experts == cores
H = 512          # expert hidden dim
KD = D // P      # k-chunks over D
KH = H // P      # k-chunks over H
MD = D // P      # output d-tiles
TB = 512         # max token block (PSUM bank / fp32 moving-operand limit)

F32 = mybir.dt.float32
F32R = mybir.dt.float32r
AF = mybir.ActivationFunctionType
ALU = mybir.AluOpType

_programs = {}
last_exec_ns = None


def _token_blocks(C):
    # Near-equal blocks of at most TB tokens (multiples of 128). Keeping
    # every block >= 256 tokens holds fp32r matmuls at full rate.
    chunks = C // P
    nb = -(-chunks // (TB // P))
    q, r = divmod(chunks, nb)
    sizes = [(q + 1) * P] * r + [q * P] * (nb - r)
    blocks = []
    t = 0
    for tb in sizes:
        blocks.append((t, tb))
        t += tb
    return blocks


def _build_program(C, repeat=1, sim_safe=False, w_dtype=F32R):
    nc = bacc.Bacc("TRN2", target_bir_lowering=False, debug=False)
    xk = nc.declare_dram_parameter("xk", [P, KD * C], F32R, isOutput=False)
    wfck = nc.declare_dram_parameter("wfck", [P, KD, H], w_dtype, isOutput=False)
    wpjk = nc.declare_dram_parameter("wpjk", [P, KH, D], w_dtype, isOutput=False)
    yk = nc.declare_dram_parameter("yk", [P, C * MD], F32, isOutput=True)

    blocks = _token_blocks(C)

    with tile.TileContext(nc) as tc:
        with (
            tc.tile_pool(name="wpool", bufs=1) as wpool,
            tc.tile_pool(name="xpool", bufs=3) as xpool,
            tc.tile_pool(name="hpool", bufs=2) as hpool,
            tc.tile_pool(name="ypool", bufs=2) as ypool,
            tc.tile_pool(name="spool", bufs=3) as spool,
            tc.tile_pool(name="hpsum", bufs=3, space="PSUM") as hpsum,
            tc.tile_pool(name="ypsum", bufs=4, space="PSUM") as ypsum,
            contextlib.ExitStack() as loop_ctx,
        ):
            wfc_sb = wpool.tile([P, KD, H], w_dtype)
            nc.sync.dma_start(wfc_sb[:], wfck[:])
            wpj_sb = wpool.tile([P, KH, D], w_dtype)
            nc.sync.dma_start(wpj_sb[:], wpjk[:])

            if repeat > 1:
                loop_ctx.enter_context(
                    tc.For_i(0, repeat, 1,
                             hint_engines=(mybir.EngineType.PE,)))

            def g1_group(x_sb, h_sb, tb, m):
                ph = hpsum.tile([P, tb], F32, tag="ph")
                for k in range(KD):
                    nc.tensor.matmul(
                        ph[:],
                        wfc_sb[:, k, m * P:(m + 1) * P],
                        x_sb[:, k, :],
                        start=(k == 0),
                        stop=(k == KD - 1),
                    )
                # h = leaky_relu_0.5(ph)^2 = (0.5*(relu(ph) + ph))^2
                # (ACT Lrelu's alpha operand produces wrong results on HW,
                # so use this 3-op form: Relu -> fused add -> Square.)
                r_sb = spool.tile([P, tb], F32, tag="r")
                nc.scalar.activation(r_sb[:], ph[:], AF.Relu)
                s_sb = spool.tile([P, tb], F32, tag="s")
                nc.vector.scalar_tensor_tensor(
                    s_sb[:], r_sb[:], 0.0, ph[:], ALU.add, ALU.add)
                nc.scalar.activation(
                    h_sb[:, m, :], s_sb[:], AF.Square, scale=0.5)

            def g2_group(h_sb, y_blk, tb, j):
                py = ypsum.tile([P, tb], F32, tag="py")
                for kh in range(KH):
                    nc.tensor.matmul(
                        py[:],
                        wpj_sb[:, kh, j * P:(j + 1) * P],
                        h_sb[:, kh, :],
                        start=(kh == 0),
                        stop=(kh == KH - 1),
                    )
                nc.vector.tensor_copy(y_blk[:, :, j], py[:])

            def y_store(t0, tb, y_blk):
                dst = yk[:, MD * t0:MD * (t0 + tb)].rearrange(
                    "p (t j) -> p t j", j=MD)
                nc.sync.dma_start(dst, y_blk[:])

            # Software pipeline with fine interleave: between the m-groups
            # of GEMM1(b), emit the j-groups of GEMM2(b-1), so the PE always
            # has independent work while block b's epilogue runs.
            prev = None
            for (t0, tb) in blocks:
                x_sb = xpool.tile([P, KD, tb], F32R, tag="x")
                src = xk[:, KD * t0:KD * (t0 + tb)].rearrange(
                    "p (k t) -> p k t", k=KD)
                nc.sync.dma_start(x_sb[:], src)
                h_sb = hpool.tile([P, KH, tb], F32R, tag="h")
                if prev is not None:
                    p0, ptb, ph_sb = prev
                    y_blk = ypool.tile([P, ptb, MD], F32, tag="y")
                for m in range(KH):
                    g1_group(x_sb, h_sb, tb, m)
                    if prev is not None:
                        g2_group(ph_sb, y_blk, ptb, 2 * m)
                        g2_group(ph_sb, y_blk, ptb, 2 * m + 1)
                if prev is not None:
                    y_store(p0, ptb, y_blk)
                prev = (t0, tb, h_sb)
            p0, ptb, ph_sb = prev
            y_blk = ypool.tile([P, ptb, MD], F32, tag="y")
            for j in range(MD):
                g2_group(ph_sb, y_blk, ptb, j)
            y_store(p0, ptb, y_blk)

    nc.compile()
    return nc


def _program(C):
    if C not in _programs:
        _programs[C] = _build_program(C)
    return _programs[C]


def _pack_x(xg, C):
    """[C, D] tokens -> block-major [P, KD*C] f32 array."""
    parts = []
    for (t0, tb) in _token_blocks(C):
        blk = xg[t0:t0 + tb].reshape(tb, KD, P).transpose(2, 1, 0)
        parts.append(blk.reshape(P, KD * tb))
    return np.ascontiguousarray(np.concatenate(parts, axis=1))


def _pack_wfc(wfc_e):
    return np.ascontiguousarray(wfc_e.T.reshape(KD, P, H).transpose(1, 0, 2))


def _pack_wproj(wproj_e):
    return np.ascontiguousarray(wproj_e.T.reshape(KH, P, D).transpose(1, 0, 2))


def _unpack_y(yk_arr, C):
    """[P, C*MD] -> [C, D]."""
    return yk_arr.reshape(P, C, MD).transpose(1, 2, 0).reshape(C, D)


def kernel(x, w_router, w_fc, w_proj):
    global last_exec_ns
    x = np.asarray(x, dtype=np.float32)
    w_router = np.asarray(w_router, dtype=np.float32)
    w_fc = np.asarray(w_fc, dtype=np.float32)
    w_proj = np.asarray(w_proj, dtype=np.float32)

    B, S, _ = x.shape
    N = B * S
    xf = np.ascontiguousarray(x.reshape(N, D))

    # Host-side router: top-1 expert per token (softmax is monotone, so
    # argmax over logits == argmax over softmax weights).
    logits = xf @ w_router.T
    eidx = np.argmax(logits, axis=1)
    counts = np.bincount(eidx, minlength=E)
    order = np.argsort(eidx, kind="stable")
    offs = np.concatenate(([0], np.cumsum(counts)))

    C = max(P, -(-int(counts.max()) // P) * P)  # round up to 128

    in_maps = []
    tok_ids = []
    for e in range(E):
        ids = order[offs[e]:offs[e + 1]]
        tok_ids.append(ids)
        xg = np.zeros((C, D), np.float32)
        xg[:len(ids)] = xf[ids]
        in_maps.append({
            "xk": _pack_x(xg, C),
            "wfck": _pack_wfc(w_fc[e]),
            "wpjk": _pack_wproj(w_proj[e]),
        })

    nc = _program(C)
    res = run_bass_kernel_spmd(nc, in_maps, core_ids=list(range(E)))
    last_exec_ns = res.exec_time_ns

    out = np.zeros((N, D), np.float32)
    for e in range(E):
        yg = _unpack_y(np.asarray(res.results[e]["yk"]), C)
        out[tok_ids[e]] = yg[:counts[e]]
    return out.reshape(B, S, D)



# revision 2
# speedup vs baseline: 1.1136x; 1.1136x over previous
"""MoE MLP (top-1 routing) on 8 TRN2 NeuronCores — bf16, low-ramp pipeline.

Expert-parallel: host computes the router argmax (replicating the reference's
jax-CPU fp32 chain exactly) and dispatches each token to its expert's core.
Core e holds w_fc[e]/w_proj[e] and runs
    y = (0.5*(z + relu(z)))^2 @ w_proj[e].T,   z = x @ w_fc[e].T
on its gathered tokens, feature-major, no on-device transposes.

All DRAM traffic is bf16 (y upcast to fp32 on the host): ~10.7MB per core,
fully under the ~58us PE roofline. Ramp/drain are minimized because the
repeat-loop (and a single launch) pays them in full:
  - token blocks [256, 512, 512, 512, 256, 128]: small first block -> PE
    starts ~1.9us after launch; small last block -> short drain.
  - x loads ride the SP HWDGE queue; weight loads and y stores ride the
    Activation HWDGE queue, so x(b1) is never stuck behind weight DMA.
  - w_fc is loaded as four m-column tiles so the first GEMM1 group only
    waits for 0.8us of weight DMA.
GEMM2 of block b-1 is interleaved between the m-groups of GEMM1 of block b,
so the PE never waits on the ACT/DVE activation chain.
"""

import contextlib
import numpy as np
import ml_dtypes

import concourse.mybir as mybir
import concourse.tile as tile
from concourse import bacc
from concourse.bass_utils import run_bass_kernel_spmd

P = 128          # SBUF partitions / PE array dim
D = 1024         # model dim
E = 8            # experts == cores
H = 512          # expert hidden dim
KD = D // P      # k-chunks over D
KH = H // P      # k-chunks over H
MD = D // P      # output d-tiles
TB = 512         # max token block (PSUM bank limit)

F32 = mybir.dt.float32
BF16 = mybir.dt.bfloat16
AF = mybir.ActivationFunctionType
ALU = mybir.AluOpType
BF16NP = ml_dtypes.bfloat16

_programs = {}
last_exec_ns = None


def _token_blocks(C):
    """Blocks of <=512 tokens: small first (fast ramp) and last (fast drain)."""
    chunks = C // P
    # schedule in 128-token chunks: [2, 4, 4, ..., 2, 1]
    if chunks <= 4:
        sizes = [chunks * P]
    else:
        sizes = [2]
        rest = chunks - 2
        # trailing small blocks: 1 chunk last, 2 chunks before it when possible
        tail = [1] if rest >= 1 else []
        rest -= sum(tail)
        mid = []
        while rest > 4:
            mid.append(4)
            rest -= 4
        if rest:
            mid.append(rest)
        # keep mid sorted descending so the smaller mid block sits near the end
        sizes = [2] + sorted(mid, reverse=True) + tail
        sizes = [s * P for s in sizes]
    blocks = []
    t = 0
    for tb in sizes:
        blocks.append((t, tb))
        t += tb
    return blocks


def _build_program(C, repeat=1):
    nc = bacc.Bacc("TRN2", target_bir_lowering=False, debug=False)
    xk = nc.declare_dram_parameter("xk", [P, KD * C], BF16, isOutput=False)
    # wfck m-major: [P, KH(m), KD(k), 128] so each m-tile is contiguous
    wfck = nc.declare_dram_parameter("wfck", [P, KH, KD, P], BF16, isOutput=False)
    wpjk = nc.declare_dram_parameter("wpjk", [P, KH, D], BF16, isOutput=False)
    yk = nc.declare_dram_parameter("yk", [P, C * MD], BF16, isOutput=True)

    blocks = _token_blocks(C)

    with tile.TileContext(nc) as tc:
        with (
            tc.tile_pool(name="wpool", bufs=1) as wpool,
            tc.tile_pool(name="xpool", bufs=3) as xpool,
            tc.tile_pool(name="hpool", bufs=2) as hpool,
            tc.tile_pool(name="ypool", bufs=2) as ypool,
            tc.tile_pool(name="spool", bufs=3) as spool,
            tc.tile_pool(name="hpsum", bufs=3, space="PSUM") as hpsum,
            tc.tile_pool(name="ypsum", bufs=4, space="PSUM") as ypsum,
            contextlib.ExitStack() as loop_ctx,
        ):
            # weight loads on the ACT HWDGE queue (x rides SP); wfc m-tiles
            # first so GEMM1(m=0) can start after ~0.8us.
            wfc_m = []
            for m in range(KH):
                wt = wpool.tile([P, KD, P], BF16, tag=f"wfc{m}")
                nc.scalar.dma_start(wt[:], wfck[:, m])
                wfc_m.append(wt)
            wpj_sb = wpool.tile([P, KH, D], BF16)
            nc.scalar.dma_start(wpj_sb[:], wpjk[:])

            # For_i pays a full all-engine barrier (plus DMA-completion
            # latency on the head x load and tail y store) at every back
            # edge. Chaining TWO full passes per trip halves that artifact
            # while keeping `repeat` == number of logical passes.
            passes = 1
            if repeat > 1:
                if repeat % 2 == 0:
                    trips, passes = repeat // 2, 2
                else:
                    trips, passes = repeat, 1
                loop_ctx.enter_context(
                    tc.For_i(0, trips, 1,
                             hint_engines=(mybir.EngineType.PE,)))

            def g1_group(x_sb, h_sb, tb, m):
                ph = hpsum.tile([P, tb], F32, tag="ph")
                for k in range(KD):
                    nc.tensor.matmul(
                        ph[:],
                        wfc_m[m][:, k, :],
                        x_sb[:, k, :],
                        start=(k == 0),
                        stop=(k == KD - 1),
                    )
                # h = leaky_relu_0.5(ph)^2 = (0.5*(relu(ph) + ph))^2
                # (ACT Lrelu's alpha operand produces wrong results on HW,
                # so use this 3-op form: Relu -> fused add -> Square.)
                r_sb = spool.tile([P, tb], F32, tag="r")
                nc.scalar.activation(r_sb[:], ph[:], AF.Relu)
                s_sb = spool.tile([P, tb], F32, tag="s")
                nc.vector.scalar_tensor_tensor(
                    s_sb[:], r_sb[:], 0.0, ph[:], ALU.add, ALU.add)
                nc.scalar.activation(
                    h_sb[:, m, :], s_sb[:], AF.Square, scale=0.5)

            def g2_group(h_sb, y_blk, tb, j):
                py = ypsum.tile([P, tb], F32, tag="py")
                for kh in range(KH):
                    nc.tensor.matmul(
                        py[:],
                        wpj_sb[:, kh, j * P:(j + 1) * P],
                        h_sb[:, kh, :],
                        start=(kh == 0),
                        stop=(kh == KH - 1),
                    )
                nc.vector.tensor_copy(y_blk[:, j, :], py[:])

            def y_store(t0, tb, y_blk):
                # y DRAM layout j-major within the block: contiguous per
                # partition; store on the ACT HWDGE queue.
                dst = yk[:, MD * t0:MD * (t0 + tb)].rearrange(
                    "p (j t) -> p j t", j=MD)
                nc.scalar.dma_start(dst, y_blk[:])

            # Software pipeline with fine interleave: between the m-groups
            # of GEMM1(b), emit the j-groups of GEMM2(b-1), so the PE always
            # has independent work while block b's epilogue runs.
            prev = None
            for (t0, tb) in blocks * passes:
                x_sb = xpool.tile([P, KD, tb], BF16, tag="x")
                src = xk[:, KD * t0:KD * (t0 + tb)].rearrange(
                    "p (k t) -> p k t", k=KD)
                nc.sync.dma_start(x_sb[:], src)
                h_sb = hpool.tile([P, KH, tb], BF16, tag="h")
                if prev is not None:
                    p0, ptb, ph_sb = prev
                    y_blk = ypool.tile([P, MD, ptb], BF16, tag="y")
                for m in range(KH):
                    g1_group(x_sb, h_sb, tb, m)
                    if prev is not None:
                        g2_group(ph_sb, y_blk, ptb, 2 * m)
                        g2_group(ph_sb, y_blk, ptb, 2 * m + 1)
                if prev is not None:
                    y_store(p0, ptb, y_blk)
                prev = (t0, tb, h_sb)
            # Drain: last block's GEMM2 with the y store split in half so the
            # first half's DMA overlaps the second half's copies.
            p0, ptb, ph_sb = prev
            half = MD // 2
            y_lo = ypool.tile([P, half, ptb], BF16, tag="ylo")
            y_hi = ypool.tile([P, half, ptb], BF16, tag="yhi")
            for j in range(MD):
                blk = y_lo if j < half else y_hi
                py = ypsum.tile([P, ptb], F32, tag="py")
                for kh in range(KH):
                    nc.tensor.matmul(
                        py[:],
                        wpj_sb[:, kh, j * P:(j + 1) * P],
                        ph_sb[:, kh, :],
                        start=(kh == 0),
                        stop=(kh == KH - 1),
                    )
                nc.vector.tensor_copy(blk[:, j % half, :], py[:])
                if j == half - 1:
                    dst = yk[:, MD * p0:MD * p0 + half * ptb].rearrange(
                        "p (j t) -> p j t", j=half)
                    nc.scalar.dma_start(dst, y_lo[:])
            dst = yk[:, MD * p0 + half * ptb:MD * (p0 + ptb)].rearrange(
                "p (j t) -> p j t", j=half)
            nc.scalar.dma_start(dst, y_hi[:])

    nc.compile()
    return nc


def _program(C):
    if C not in _programs:
        _programs[C] = _build_program(C)
    return _programs[C]


def _pack_x(xg, C):
    """[C, D] bf16 tokens -> block-major [P, KD*C] bf16 array."""
    parts = []
    for (t0, tb) in _token_blocks(C):
        blk = xg[t0:t0 + tb].reshape(tb, KD, P).transpose(2, 1, 0)
        parts.append(blk.reshape(P, KD * tb))
    return np.ascontiguousarray(np.concatenate(parts, axis=1))


def _pack_wfc(wfc_e):
    # [H, D] -> [P(d_low), KH(m), KD(k), P(h_low)]:
    # [p, m, k, j] = wfc_e[m*128+j, k*128+p]
    w = wfc_e.reshape(KH, P, KD, P)          # [m, j, k, p]
    return np.ascontiguousarray(w.transpose(3, 0, 2, 1).astype(BF16NP))


def _pack_wproj(wproj_e):
    return np.ascontiguousarray(
        wproj_e.T.reshape(KH, P, D).transpose(1, 0, 2).astype(BF16NP))


def _unpack_y(yk_arr, C):
    """bf16 [P, C*MD] (block-major, j-major in block) -> fp32 [C, D]."""
    out = np.empty((C, D), np.float32)
    for (t0, tb) in _token_blocks(C):
        blk = yk_arr[:, MD * t0:MD * (t0 + tb)].reshape(P, MD, tb)
        out[t0:t0 + tb] = blk.transpose(2, 1, 0).reshape(tb, D)
    return out


def _route(x, w_router):
    """Replicate the reference router bit-exactly on jax-CPU fp32."""
    import jax
    import jax.numpy as jnp

    cpu = jax.devices("cpu")[0]
    with jax.default_device(cpu):
        logits = jnp.einsum('bsd,ed->bse', jnp.asarray(x), jnp.asarray(w_router))
        weights = jax.nn.softmax(logits, axis=-1)
        idx = np.asarray(jnp.argmax(weights, axis=-1))
    return idx.reshape(-1)


def make_in_maps(xf, eidx, C, w_fc, w_proj):
    counts = np.bincount(eidx, minlength=E)
    order = np.argsort(eidx, kind="stable")
    offs = np.concatenate(([0], np.cumsum(counts)))
    xb = xf.astype(BF16NP)
    in_maps = []
    tok_ids = []
    for e in range(E):
        ids = order[offs[e]:offs[e + 1]]
        tok_ids.append(ids)
        xg = np.zeros((C, D), BF16NP)
        xg[:len(ids)] = xb[ids]
        in_maps.append({
            "xk": _pack_x(xg, C),
            "wfck": _pack_wfc(w_fc[e]),
            "wpjk": _pack_wproj(w_proj[e]),
        })
    return in_maps, tok_ids


def kernel(x, w_router, w_fc, w_proj):
    global last_exec_ns
    x = np.asarray(x, dtype=np.float32)
    w_router = np.asarray(w_router, dtype=np.float32)
    w_fc = np.asarray(w_fc, dtype=np.float32)
    w_proj = np.asarray(w_proj, dtype=np.float32)

    B, S, _ = x.shape
    N = B * S
    xf = np.ascontiguousarray(x.reshape(N, D))

    eidx = _route(x, w_router)
    counts = np.bincount(eidx, minlength=E)

    C = max(P, -(-int(counts.max()) // P) * P)  # round up to 128

    in_maps, tok_ids = make_in_maps(xf, eidx, C, w_fc, w_proj)

    nc = _program(C)
    res = run_bass_kernel_spmd(nc, in_maps, core_ids=list(range(E)))
    last_exec_ns = res.exec_time_ns

    out = np.zeros((N, D), np.float32)
    for e in range(E):
        yg = _unpack_y(np.asarray(res.results[e]["yk"]), C)
        out[tok_ids[e]] = yg[:counts[e]]
    return out.reshape(B, S, D)


# revision 3
# speedup vs baseline: 1.1390x; 1.0228x over previous
"""MoE MLP (top-1 routing) on 8 TRN2 NeuronCores — bf16, low-ramp pipeline.

Expert-parallel: host computes the router argmax (replicating the reference's
jax-CPU fp32 chain exactly) and dispatches each token to its expert's core.
Core e holds w_fc[e]/w_proj[e] and runs
    y = (0.5*(z + relu(z)))^2 @ w_proj[e].T,   z = x @ w_fc[e].T
on its gathered tokens, feature-major, no on-device transposes.

All DRAM traffic is bf16 (y upcast to fp32 on the host): ~10.7MB per core,
fully under the ~58us PE roofline. Ramp/drain are minimized because the
repeat-loop (and a single launch) pays them in full:
  - token blocks [256, 512, 512, 512, 256, 128]: small first block -> PE
    starts ~1.9us after launch; small last block -> short drain.
  - x loads ride the SP HWDGE queue; weight loads and y stores ride the
    Activation HWDGE queue, so x(b1) is never stuck behind weight DMA.
  - w_fc is loaded as four m-column tiles so the first GEMM1 group only
    waits for 0.8us of weight DMA.
GEMM2 of block b-1 is interleaved between the m-groups of GEMM1 of block b,
so the PE never waits on the ACT/DVE activation chain.
"""

import contextlib
import numpy as np
import ml_dtypes

import concourse.mybir as mybir
import concourse.tile as tile
from concourse import bacc
from concourse.bass_utils import run_bass_kernel_spmd

P = 128          # SBUF partitions / PE array dim
D = 1024         # model dim
E = 8            # experts == cores
H = 512          # expert hidden dim
KD = D // P      # k-chunks over D
KH = H // P      # k-chunks over H
MD = D // P      # output d-tiles
TB = 512         # max token block (PSUM bank limit)

F32 = mybir.dt.float32
BF16 = mybir.dt.bfloat16
AF = mybir.ActivationFunctionType
ALU = mybir.AluOpType
BF16NP = ml_dtypes.bfloat16

_programs = {}
last_exec_ns = None


def _token_blocks(C):
    """Blocks of <=512 tokens: small first (fast ramp) and last (fast drain)."""
    chunks = C // P
    # schedule in 128-token chunks: [2, 4, 4, ..., 2, 1]
    if chunks <= 4:
        sizes = [chunks * P]
    else:
        sizes = [2]
        rest = chunks - 2
        # trailing small blocks: 1 chunk last, 2 chunks before it when possible
        tail = [1] if rest >= 1 else []
        rest -= sum(tail)
        mid = []
        while rest > 4:
            mid.append(4)
            rest -= 4
        if rest:
            mid.append(rest)
        # keep mid sorted descending so the smaller mid block sits near the end
        sizes = [2] + sorted(mid, reverse=True) + tail
        sizes = [s * P for s in sizes]
    blocks = []
    t = 0
    for tb in sizes:
        blocks.append((t, tb))
        t += tb
    return blocks


def _build_program(C, repeat=1):
    nc = bacc.Bacc("TRN2", target_bir_lowering=False, debug=False)
    xk = nc.declare_dram_parameter("xk", [P, KD * C], BF16, isOutput=False)
    # wfck m-major: [P, KH(m), KD(k), 128] so each m-tile is contiguous
    wfck = nc.declare_dram_parameter("wfck", [P, KH, KD, P], BF16, isOutput=False)
    wpjk = nc.declare_dram_parameter("wpjk", [P, KH, D], BF16, isOutput=False)
    yk = nc.declare_dram_parameter("yk", [P, C * MD], BF16, isOutput=True)

    blocks = _token_blocks(C)

    with tile.TileContext(nc) as tc:
        with (
            tc.tile_pool(name="wpool", bufs=1) as wpool,
            tc.tile_pool(name="xpool", bufs=3) as xpool,
            tc.tile_pool(name="hpool", bufs=2) as hpool,
            tc.tile_pool(name="ypool", bufs=2) as ypool,
            tc.tile_pool(name="spool", bufs=3) as spool,
            tc.tile_pool(name="hpsum", bufs=3, space="PSUM") as hpsum,
            tc.tile_pool(name="ypsum", bufs=4, space="PSUM") as ypsum,
            contextlib.ExitStack() as loop_ctx,
        ):
            # weight loads on the ACT HWDGE queue (x rides SP); wfc m-tiles
            # first so GEMM1(m=0) can start after ~0.8us.
            wfc_m = []
            for m in range(KH):
                wt = wpool.tile([P, KD, P], BF16, tag=f"wfc{m}")
                nc.scalar.dma_start(wt[:], wfck[:, m])
                wfc_m.append(wt)
            wpj_sb = wpool.tile([P, KH, D], BF16)
            nc.scalar.dma_start(wpj_sb[:], wpjk[:])

            # For_i pays a full all-engine barrier (plus DMA-completion
            # latency on the head x load and tail y store) at every back
            # edge. Chaining TWO full passes per trip halves that artifact
            # while keeping `repeat` == number of logical passes.
            passes = 1
            if repeat > 1:
                for p in (4, 3, 2):
                    if repeat % p == 0:
                        passes = p
                        break
                trips = repeat // passes
                loop_ctx.enter_context(
                    tc.For_i(0, trips, 1,
                             hint_engines=(mybir.EngineType.PE,)))

            def g1_group(x_sb, h_sb, tb, m):
                ph = hpsum.tile([P, tb], F32, tag="ph")
                for k in range(KD):
                    nc.tensor.matmul(
                        ph[:],
                        wfc_m[m][:, k, :],
                        x_sb[:, k, :],
                        start=(k == 0),
                        stop=(k == KD - 1),
                    )
                # h = leaky_relu_0.5(ph)^2 = (0.5*(relu(ph) + ph))^2
                # (ACT Lrelu's alpha operand produces wrong results on HW,
                # so use this 3-op form: Relu -> fused add -> Square.)
                r_sb = spool.tile([P, tb], BF16, tag="r")
                nc.scalar.activation(r_sb[:], ph[:], AF.Relu)
                s_sb = spool.tile([P, tb], F32, tag="s")
                nc.vector.scalar_tensor_tensor(
                    s_sb[:], r_sb[:], 0.0, ph[:], ALU.add, ALU.add)
                nc.scalar.activation(
                    h_sb[:, m, :], s_sb[:], AF.Square, scale=0.5)

            def g2_group(h_sb, y_blk, tb, j):
                py = ypsum.tile([P, tb], F32, tag="py")
                for kh in range(KH):
                    nc.tensor.matmul(
                        py[:],
                        wpj_sb[:, kh, j * P:(j + 1) * P],
                        h_sb[:, kh, :],
                        start=(kh == 0),
                        stop=(kh == KH - 1),
                    )
                # PSUM->SBUF drains alternate between DVE and ACT so neither
                # engine serializes the whole y path.
                if j % 2 == 0:
                    nc.vector.tensor_copy(y_blk[:, j, :], py[:])
                else:
                    nc.scalar.activation(y_blk[:, j, :], py[:], AF.Copy)

            def y_store(t0, tb, y_blk):
                # y DRAM layout j-major within the block: contiguous per
                # partition; store on the Pool SWDGE queue, which is otherwise
                # idle — the issuing engine is blocked for the transfer, so
                # keeping stores off SP (x loads) and ACT (activations) frees
                # both.
                dst = yk[:, MD * t0:MD * (t0 + tb)].rearrange(
                    "p (j t) -> p j t", j=MD)
                nc.gpsimd.dma_start(dst, y_blk[:])

            # Software pipeline with fine interleave: between the m-groups
            # of GEMM1(b), emit the j-groups of GEMM2(b-1), so the PE always
            # has independent work while block b's epilogue runs.
            prev = None
            for (t0, tb) in blocks * passes:
                x_sb = xpool.tile([P, KD, tb], BF16, tag="x")
                src = xk[:, KD * t0:KD * (t0 + tb)].rearrange(
                    "p (k t) -> p k t", k=KD)
                nc.sync.dma_start(x_sb[:], src)
                h_sb = hpool.tile([P, KH, tb], BF16, tag="h")
                if prev is not None:
                    p0, ptb, ph_sb = prev
                    y_blk = ypool.tile([P, MD, ptb], BF16, tag="y")
                for m in range(KH):
                    g1_group(x_sb, h_sb, tb, m)
                    if prev is not None:
                        g2_group(ph_sb, y_blk, ptb, 2 * m)
                        g2_group(ph_sb, y_blk, ptb, 2 * m + 1)
                if prev is not None:
                    y_store(p0, ptb, y_blk)
                prev = (t0, tb, h_sb)
            # Drain: last block's GEMM2 with the y store split in half so the
            # first half's DMA overlaps the second half's copies.
            p0, ptb, ph_sb = prev
            half = MD // 2
            y_lo = ypool.tile([P, half, ptb], BF16, tag="ylo")
            y_hi = ypool.tile([P, half, ptb], BF16, tag="yhi")
            for j in range(MD):
                blk = y_lo if j < half else y_hi
                py = ypsum.tile([P, ptb], F32, tag="py")
                for kh in range(KH):
                    nc.tensor.matmul(
                        py[:],
                        wpj_sb[:, kh, j * P:(j + 1) * P],
                        ph_sb[:, kh, :],
                        start=(kh == 0),
                        stop=(kh == KH - 1),
                    )
                if j % 2 == 0:
                    nc.vector.tensor_copy(blk[:, j % half, :], py[:])
                else:
                    nc.scalar.activation(blk[:, j % half, :], py[:], AF.Copy)
                if j == half - 1:
                    dst = yk[:, MD * p0:MD * p0 + half * ptb].rearrange(
                        "p (j t) -> p j t", j=half)
                    nc.gpsimd.dma_start(dst, y_lo[:])
            dst = yk[:, MD * p0 + half * ptb:MD * (p0 + ptb)].rearrange(
                "p (j t) -> p j t", j=half)
            nc.gpsimd.dma_start(dst, y_hi[:])

    nc.compile()
    return nc


def _program(C):
    if C not in _programs:
        _programs[C] = _build_program(C)
    return _programs[C]


def _pack_x(xg, C):
    """[C, D] bf16 tokens -> block-major [P, KD*C] bf16 array."""
    parts = []
    for (t0, tb) in _token_blocks(C):
        blk = xg[t0:t0 + tb].reshape(tb, KD, P).transpose(2, 1, 0)
        parts.append(blk.reshape(P, KD * tb))
    return np.ascontiguousarray(np.concatenate(parts, axis=1))


def _pack_wfc(wfc_e):
    # [H, D] -> [P(d_low), KH(m), KD(k), P(h_low)]:
    # [p, m, k, j] = wfc_e[m*128+j, k*128+p]
    w = wfc_e.reshape(KH, P, KD, P)          # [m, j, k, p]
    return np.ascontiguousarray(w.transpose(3, 0, 2, 1).astype(BF16NP))


def _pack_wproj(wproj_e):
    return np.ascontiguousarray(
        wproj_e.T.reshape(KH, P, D).transpose(1, 0, 2).astype(BF16NP))


def _unpack_y(yk_arr, C):
    """bf16 [P, C*MD] (block-major, j-major in block) -> fp32 [C, D]."""
    out = np.empty((C, D), np.float32)
    for (t0, tb) in _token_blocks(C):
        blk = yk_arr[:, MD * t0:MD * (t0 + tb)].reshape(P, MD, tb)
        out[t0:t0 + tb] = blk.transpose(2, 1, 0).reshape(tb, D)
    return out


def _route(x, w_router):
    """Replicate the reference router bit-exactly on jax-CPU fp32."""
    import jax
    import jax.numpy as jnp

    cpu = jax.devices("cpu")[0]
    with jax.default_device(cpu):
        logits = jnp.einsum('bsd,ed->bse', jnp.asarray(x), jnp.asarray(w_router))
        weights = jax.nn.softmax(logits, axis=-1)
        idx = np.asarray(jnp.argmax(weights, axis=-1))
    return idx.reshape(-1)


def make_in_maps(xf, eidx, C, w_fc, w_proj):
    counts = np.bincount(eidx, minlength=E)
    order = np.argsort(eidx, kind="stable")
    offs = np.concatenate(([0], np.cumsum(counts)))
    xb = xf.astype(BF16NP)
    in_maps = []
    tok_ids = []
    for e in range(E):
        ids = order[offs[e]:offs[e + 1]]
        tok_ids.append(ids)
        xg = np.zeros((C, D), BF16NP)
        xg[:len(ids)] = xb[ids]
        in_maps.append({
            "xk": _pack_x(xg, C),
            "wfck": _pack_wfc(w_fc[e]),
            "wpjk": _pack_wproj(w_proj[e]),
        })
    return in_maps, tok_ids


def kernel(x, w_router, w_fc, w_proj):
    global last_exec_ns
    x = np.asarray(x, dtype=np.float32)
    w_router = np.asarray(w_router, dtype=np.float32)
    w_fc = np.asarray(w_fc, dtype=np.float32)
    w_proj = np.asarray(w_proj, dtype=np.float32)

    B, S, _ = x.shape
    N = B * S
    xf = np.ascontiguousarray(x.reshape(N, D))

    eidx = _route(x, w_router)
    counts = np.bincount(eidx, minlength=E)

    C = max(P, -(-int(counts.max()) // P) * P)  # round up to 128

    in_maps, tok_ids = make_in_maps(xf, eidx, C, w_fc, w_proj)

    nc = _program(C)
    res = run_bass_kernel_spmd(nc, in_maps, core_ids=list(range(E)))
    last_exec_ns = res.exec_time_ns

    out = np.zeros((N, D), np.float32)
    for e in range(E):
        yg = _unpack_y(np.asarray(res.results[e]["yk"]), C)
        out[tok_ids[e]] = yg[:counts[e]]
    return out.reshape(B, S, D)


# revision 4
# speedup vs baseline: 1.2054x; 1.0583x over previous
"""MoE MLP (top-1 routing) on 8 TRN2 NeuronCores — bf16, low-ramp pipeline.

Expert-parallel: host computes the router argmax (replicating the reference's
jax-CPU fp32 chain exactly) and dispatches each token to its expert's core.
Core e holds w_fc[e]/w_proj[e] and runs
    y = (0.5*(z + relu(z)))^2 @ w_proj[e].T,   z = x @ w_fc[e].T
on its gathered tokens, feature-major, no on-device transposes.

All DRAM traffic is bf16 (y upcast to fp32 on the host): ~10.7MB per core,
fully under the ~58us PE roofline. Ramp/drain are minimized because the
repeat-loop (and a single launch) pays them in full:
  - token blocks [256, 512, 512, 512, 256, 128]: small first block -> PE
    starts ~1.9us after launch; small last block -> short drain.
  - x loads ride the SP HWDGE queue; weight loads and y stores ride the
    Activation HWDGE queue, so x(b1) is never stuck behind weight DMA.
  - w_fc is loaded as four m-column tiles so the first GEMM1 group only
    waits for 0.8us of weight DMA.
GEMM2 of block b-1 is interleaved between the m-groups of GEMM1 of block b,
so the PE never waits on the ACT/DVE activation chain.
"""

import contextlib
import numpy as np
import ml_dtypes

import concourse.mybir as mybir
import concourse.tile as tile
from concourse import bacc
from concourse.bass_utils import run_bass_kernel_spmd

P = 128          # SBUF partitions / PE array dim
D = 1024         # model dim
E = 8            # experts == cores
H = 512          # expert hidden dim
KD = D // P      # k-chunks over D
KH = H // P      # k-chunks over H
MD = D // P      # output d-tiles
TB = 512         # max token block (PSUM bank limit)

F32 = mybir.dt.float32
BF16 = mybir.dt.bfloat16
AF = mybir.ActivationFunctionType
ALU = mybir.AluOpType
BF16NP = ml_dtypes.bfloat16

_programs = {}
last_exec_ns = None


def _token_blocks(C):
    """Blocks of <=512 tokens: small first (fast ramp) and last (fast drain)."""
    chunks = C // P
    # schedule in 128-token chunks: [2, 4, 4, ..., 2, 1]
    if chunks <= 4:
        sizes = [chunks * P]
    else:
        sizes = [2]
        rest = chunks - 2
        # trailing small blocks: 1 chunk last, 2 chunks before it when possible
        tail = [1] if rest >= 1 else []
        rest -= sum(tail)
        mid = []
        while rest > 4:
            mid.append(4)
            rest -= 4
        if rest:
            mid.append(rest)
        # keep mid sorted descending so the smaller mid block sits near the end
        sizes = [2] + sorted(mid, reverse=True) + tail
        sizes = [s * P for s in sizes]
    blocks = []
    t = 0
    for tb in sizes:
        blocks.append((t, tb))
        t += tb
    return blocks


def _build_program(C, repeat=1):
    nc = bacc.Bacc("TRN2", target_bir_lowering=False, debug=False)
    xk = nc.declare_dram_parameter("xk", [P, KD * C], BF16, isOutput=False)
    # wfck m-major: [P, KH(m), KD(k), 128] so each m-tile is contiguous
    wfck = nc.declare_dram_parameter("wfck", [P, KH, KD, P], BF16, isOutput=False)
    wpjk = nc.declare_dram_parameter("wpjk", [P, KH, D], BF16, isOutput=False)
    yk = nc.declare_dram_parameter("yk", [P, C * MD], BF16, isOutput=True)

    blocks = _token_blocks(C)

    with tile.TileContext(nc) as tc:
        with (
            tc.tile_pool(name="wpool", bufs=1) as wpool,
            tc.tile_pool(name="xpool", bufs=3) as xpool,
            tc.tile_pool(name="hpool", bufs=2) as hpool,
            tc.tile_pool(name="ypool", bufs=2) as ypool,
            tc.tile_pool(name="spool", bufs=3) as spool,
            tc.tile_pool(name="hpsum", bufs=3, space="PSUM") as hpsum,
            tc.tile_pool(name="ypsum", bufs=3, space="PSUM") as ypsum,
            tc.tile_pool(name="spsum", bufs=2, space="PSUM") as spsum,
            contextlib.ExitStack() as loop_ctx,
        ):
            # weight loads on the ACT HWDGE queue (x rides SP); wfc m-tiles
            # first so GEMM1(m=0) can start after ~0.8us.
            wfc_m = []
            for m in range(KH):
                wt = wpool.tile([P, KD, P], BF16, tag=f"wfc{m}")
                nc.scalar.dma_start(wt[:], wfck[:, m])
                wfc_m.append(wt)
            wpj_sb = wpool.tile([P, KH, D], BF16)
            nc.scalar.dma_start(wpj_sb[:], wpjk[:])

            # For_i pays a full all-engine barrier (plus DMA-completion
            # latency on the head x load and tail y store) at every back
            # edge. Chaining TWO full passes per trip halves that artifact
            # while keeping `repeat` == number of logical passes.
            passes = 1
            if repeat > 1:
                for p in (4, 3, 2):
                    if repeat % p == 0:
                        passes = p
                        break
                trips = repeat // passes
                loop_ctx.enter_context(
                    tc.For_i(0, trips, 1,
                             hint_engines=(mybir.EngineType.PE,)))

            def g1_group(x_sb, h_sb, tb, m):
                ph = hpsum.tile([P, tb], F32, tag="ph")
                for k in range(KD):
                    nc.tensor.matmul(
                        ph[:],
                        wfc_m[m][:, k, :],
                        x_sb[:, k, :],
                        start=(k == 0),
                        stop=(k == KD - 1),
                    )
                # h = leaky_relu_0.5(ph)^2 = (0.5*(relu(ph) + ph))^2
                # (ACT Lrelu's alpha operand produces wrong results on HW,
                # so use this 3-op form: Relu -> fused add -> Square.)
                r_sb = spool.tile([P, tb], BF16, tag="r")
                nc.scalar.activation(r_sb[:], ph[:], AF.Relu)
                s_sb = spsum.tile([P, tb], F32, tag="s")
                nc.vector.scalar_tensor_tensor(
                    s_sb[:], r_sb[:], 0.0, ph[:], ALU.add, ALU.add)
                nc.scalar.activation(
                    h_sb[:, m, :], s_sb[:], AF.Square, scale=0.5)

            def g2_group(h_sb, y_blk, tb, j):
                py = ypsum.tile([P, tb], F32, tag="py")
                for kh in range(KH):
                    nc.tensor.matmul(
                        py[:],
                        wpj_sb[:, kh, j * P:(j + 1) * P],
                        h_sb[:, kh, :],
                        start=(kh == 0),
                        stop=(kh == KH - 1),
                    )
                # PSUM->SBUF drains alternate between DVE and ACT so neither
                # engine serializes the whole y path.
                if j % 2 == 0:
                    nc.vector.tensor_copy(y_blk[:, j, :], py[:])
                else:
                    nc.scalar.activation(y_blk[:, j, :], py[:], AF.Copy)

            def y_store(t0, tb, y_blk):
                # y DRAM layout j-major within the block: contiguous per
                # partition; store on the Pool SWDGE queue, which is otherwise
                # idle — the issuing engine is blocked for the transfer, so
                # keeping stores off SP (x loads) and ACT (activations) frees
                # both.
                dst = yk[:, MD * t0:MD * (t0 + tb)].rearrange(
                    "p (j t) -> p j t", j=MD)
                nc.gpsimd.dma_start(dst, y_blk[:])

            # Software pipeline with fine interleave: between the m-groups
            # of GEMM1(b), emit the j-groups of GEMM2(b-1), so the PE always
            # has independent work while block b's epilogue runs.
            prev = None
            for bi, (t0, tb) in enumerate(blocks * passes):
                x_sb = xpool.tile([P, KD, tb], BF16, tag="x")
                src = xk[:, KD * t0:KD * (t0 + tb)].rearrange(
                    "p (k t) -> p k t", k=KD)
                (nc.sync if bi % 2 == 0 else nc.gpsimd).dma_start(x_sb[:], src)
                h_sb = hpool.tile([P, KH, tb], BF16, tag="h")
                if prev is not None:
                    p0, ptb, ph_sb = prev
                    y_blk = ypool.tile([P, MD, ptb], BF16, tag="y")
                for m in range(KH):
                    g1_group(x_sb, h_sb, tb, m)
                    if prev is not None:
                        g2_group(ph_sb, y_blk, ptb, 2 * m)
                        g2_group(ph_sb, y_blk, ptb, 2 * m + 1)
                if prev is not None:
                    y_store(p0, ptb, y_blk)
                prev = (t0, tb, h_sb)
            # Drain: last block's GEMM2 with the y store split in half so the
            # first half's DMA overlaps the second half's copies.
            p0, ptb, ph_sb = prev
            half = MD // 2
            y_lo = ypool.tile([P, half, ptb], BF16, tag="ylo")
            y_hi = ypool.tile([P, half, ptb], BF16, tag="yhi")
            for j in range(MD):
                blk = y_lo if j < half else y_hi
                py = ypsum.tile([P, ptb], F32, tag="py")
                for kh in range(KH):
                    nc.tensor.matmul(
                        py[:],
                        wpj_sb[:, kh, j * P:(j + 1) * P],
                        ph_sb[:, kh, :],
                        start=(kh == 0),
                        stop=(kh == KH - 1),
                    )
                if j % 2 == 0:
                    nc.vector.tensor_copy(blk[:, j % half, :], py[:])
                else:
                    nc.scalar.activation(blk[:, j % half, :], py[:], AF.Copy)
                if j == half - 1:
                    dst = yk[:, MD * p0:MD * p0 + half * ptb].rearrange(
                        "p (j t) -> p j t", j=half)
                    nc.gpsimd.dma_start(dst, y_lo[:])
            dst = yk[:, MD * p0 + half * ptb:MD * (p0 + ptb)].rearrange(
                "p (j t) -> p j t", j=half)
            nc.gpsimd.dma_start(dst, y_hi[:])

    nc.compile()
    return nc


def _program(C):
    if C not in _programs:
        _programs[C] = _build_program(C)
    return _programs[C]


def _pack_x(xg, C):
    """[C, D] bf16 tokens -> block-major [P, KD*C] bf16 array."""
    parts = []
    for (t0, tb) in _token_blocks(C):
        blk = xg[t0:t0 + tb].reshape(tb, KD, P).transpose(2, 1, 0)
        parts.append(blk.reshape(P, KD * tb))
    return np.ascontiguousarray(np.concatenate(parts, axis=1))


def _pack_wfc(wfc_e):
    # [H, D] -> [P(d_low), KH(m), KD(k), P(h_low)]:
    # [p, m, k, j] = wfc_e[m*128+j, k*128+p]
    w = wfc_e.reshape(KH, P, KD, P)          # [m, j, k, p]
    return np.ascontiguousarray(w.transpose(3, 0, 2, 1).astype(BF16NP))


def _pack_wproj(wproj_e):
    return np.ascontiguousarray(
        wproj_e.T.reshape(KH, P, D).transpose(1, 0, 2).astype(BF16NP))


def _unpack_y(yk_arr, C):
    """bf16 [P, C*MD] (block-major, j-major in block) -> fp32 [C, D]."""
    out = np.empty((C, D), np.float32)
    for (t0, tb) in _token_blocks(C):
        blk = yk_arr[:, MD * t0:MD * (t0 + tb)].reshape(P, MD, tb)
        out[t0:t0 + tb] = blk.transpose(2, 1, 0).reshape(tb, D)
    return out


def _route(x, w_router):
    """Replicate the reference router bit-exactly on jax-CPU fp32."""
    import jax
    import jax.numpy as jnp

    cpu = jax.devices("cpu")[0]
    with jax.default_device(cpu):
        logits = jnp.einsum('bsd,ed->bse', jnp.asarray(x), jnp.asarray(w_router))
        weights = jax.nn.softmax(logits, axis=-1)
        idx = np.asarray(jnp.argmax(weights, axis=-1))
    return idx.reshape(-1)


def make_in_maps(xf, eidx, C, w_fc, w_proj):
    counts = np.bincount(eidx, minlength=E)
    order = np.argsort(eidx, kind="stable")
    offs = np.concatenate(([0], np.cumsum(counts)))
    xb = xf.astype(BF16NP)
    in_maps = []
    tok_ids = []
    for e in range(E):
        ids = order[offs[e]:offs[e + 1]]
        tok_ids.append(ids)
        xg = np.zeros((C, D), BF16NP)
        xg[:len(ids)] = xb[ids]
        in_maps.append({
            "xk": _pack_x(xg, C),
            "wfck": _pack_wfc(w_fc[e]),
            "wpjk": _pack_wproj(w_proj[e]),
        })
    return in_maps, tok_ids


def kernel(x, w_router, w_fc, w_proj):
    global last_exec_ns
    x = np.asarray(x, dtype=np.float32)
    w_router = np.asarray(w_router, dtype=np.float32)
    w_fc = np.asarray(w_fc, dtype=np.float32)
    w_proj = np.asarray(w_proj, dtype=np.float32)

    B, S, _ = x.shape
    N = B * S
    xf = np.ascontiguousarray(x.reshape(N, D))

    eidx = _route(x, w_router)
    counts = np.bincount(eidx, minlength=E)

    C = max(P, -(-int(counts.max()) // P) * P)  # round up to 128

    in_maps, tok_ids = make_in_maps(xf, eidx, C, w_fc, w_proj)

    nc = _program(C)
    res = run_bass_kernel_spmd(nc, in_maps, core_ids=list(range(E)))
    last_exec_ns = res.exec_time_ns

    out = np.zeros((N, D), np.float32)
    for e in range(E):
        yg = _unpack_y(np.asarray(res.results[e]["yk"]), C)
        out[tok_ids[e]] = yg[:counts[e]]
    return out.reshape(B, S, D)
